# revision 12
# baseline (speedup 1.0000x reference)
"""Trainium2 Bass kernel for nn_AttDecoder (GRU + coverage attention decoder).

Sharding: pure data parallel — batch 8 across 8 NeuronCores (batch=1/core).

v3 structure (validated on host, rel err ~3.2e-3):
  - all static preprocessing on host: trans = enc_conv(cnn)+pos+b, K2 =
    att_weight_W @ att_conv_w, M2T = cnn^T ctx_W^T, gi = W_ih emb[words]+b,
    embw-term (+ output biases + counting ctx), hidden0. Device loads ~2MB.
  - coverage conv as one K=121 matmul per d-chunk from a [121, 1024] im2col
    gathered by ONE DMA from a padded DRAM staging buffer (row stride 84);
    trans added by identity-matmul into the same PSUM accumulation group.
  - alpha_sum kept only in bf16 ([128,8], one STT per step) -> ONE scatter DMA.
  - tanh per d-chunk on ACT ([128,1024] from PSUM, query as per-partition
    bias); softmax without max-subtraction; mask folded as exp(energy+ln m).
  - ctx/out_state/prob tail batched AFTER the decode loop (N=36 matmuls)
    using per-step e8/hbf/1-over-esum histories.
Layouts: pos = h*64+w (1024); within 128-slices pos = j*128 + 64q + w with
h = 2j+q; d in 4x128 chunks.
"""

import json
import math
import sys

import numpy as np
import ml_dtypes

sys.path.insert(0, "/opt/trn_rl_repo")

import concourse.bass as bass
import concourse.mybir as mybir
import concourse.tile as tile
from concourse.bass_utils import run_bass_kernel_spmd
from concourse.masks import make_identity

B, C, H, W = 8, 684, 16, 64
HID, INP, AD, V, T = 256, 256, 512, 111, 36
RATIO = 16
HW = H * W
ND = AD // 128
NJ = HW // 128
PSTR = 84
GLEN = 16 * PSTR  # im2col gather row length (16 h-rows x 84 stride)
P2D_LEN = 3072
BF = mybir.dt.bfloat16
F32 = mybir.dt.float32

_bf = lambda x: np.ascontiguousarray(np.asarray(x, dtype=np.float32)).astype(
    ml_dtypes.bfloat16
)
_f32 = lambda x: np.ascontiguousarray(np.asarray(x, dtype=np.float32))


def _chunk_k(a, k_pad=None):
    """[K, M] -> [128, (K/128)*M]; out[p, kc*M+m] = a[kc*128+p, m]."""
    a = np.asarray(a, dtype=np.float32)
    k, m = a.shape
    kp = k_pad or k
    if kp > k:
        a = np.concatenate([a, np.zeros((kp - k, m), np.float32)], 0)
    nk = kp // 128
    assert nk * 128 == kp
    return np.ascontiguousarray(
        a.reshape(nk, 128, m).transpose(1, 0, 2).reshape(128, nk * m)
    )


def _pos_embedding_sine(mask_hw):
    """numpy port of reference.pos_embedding_sine; [B,H,W] -> [B,512,H,W]."""
    num_pos_feats, temperature = 256, 10000.0
    scale = 2.0 * math.pi
    eps = 1e-6
    m = np.asarray(mask_hw, np.float32)
    y = np.cumsum(m, axis=1)
    x = np.cumsum(m, axis=2)
    y = y / (y[:, -1:, :] + eps) * scale
    x = x / (x[:, :, -1:] + eps) * scale
    i = np.arange(num_pos_feats, dtype=np.float32)
    dim_t = temperature ** (2.0 * np.floor(i / 2.0) / num_pos_feats)
    px = x[..., None] / dim_t
    py = y[..., None] / dim_t

    def inter(p):
        return np.stack((np.sin(p[..., 0::2]), np.cos(p[..., 1::2])), axis=4).reshape(
            p.shape[:3] + (num_pos_feats,)
        )

    pos = np.concatenate((inter(py), inter(px)), axis=3)
    return np.transpose(pos, (0, 3, 1, 2))


# ------------------------------------------------- walrus wait-split shim
def _split_sync_waits(bir_json: bytes, max_waits: int = 1) -> bytes:
    """This walrus build encodes one sem wait per instruction; hoist extras
    onto NoOps inserted before the instruction on the same engine."""
    js = json.loads(bir_json)
    n = 0
    for fn in js.get("functions", []):
        for bb in fn.get("blocks", []):
            out = []
            for ins in bb.get("instructions", []):
                si = ins.get("sync_info")
                waits = (si or {}).get("on_wait") or []
                upds = (si or {}).get("on_update") or []
                assert len(upds) <= 1, ins.get("name")
                if len(waits) > max_waits:
                    extra, si["on_wait"] = waits[:-max_waits], waits[-max_waits:]
                    for w in extra:
                        n += 1
                        out.append(
                            {
                                "debug": ins.get("debug", 0),
                                "engine": ins["engine"],
                                "ins": [],
                                "outs": [],
                                "name": f"WSPLIT-{n}",
                                "opcode": "NoOp",
                                "sync_info": {"on_wait": [w], "on_update": []},
                            }
                        )
                out.append(ins)
            bb["instructions"] = out
    return json.dumps(js).encode()


_shim_installed = False


def _install_shim():
    global _shim_installed
    if _shim_installed:
        return
    import concourse.bass2jax as bass2jax

    orig = bass2jax.compile_bir_kernel

    def wrapper(bir_json, tmpdir, neff_name="file.neff"):
        return orig(_split_sync_waits(bir_json), tmpdir, neff_name)

    bass2jax.compile_bir_kernel = wrapper
    _shim_installed = True


# ------------------------------------------------------------ bass builder
_INPUT_SPEC = {
    # small, needed first in the decode loop
    "gi_all": ([128, 6 * T], F32),
    "h0": ([128, 2], F32),
    "w_hhT": ([128, 2 * 3 * HID], BF),
    "att_hT": ([128, 2 * AD], BF),
    "lnmask_col": ([128, NJ], BF),
    "w_col": ([128, ND], BF),
    "bhn_col": ([128, 2], F32),
    "att_hb_col": ([128, ND], F32),
    "ab_col": ([128, 1], F32),
    "k2": ([121, AD], BF),
    # big per-core tensors
    "trans": ([128, ND * HW], BF),
    # tail-only
    "state_T": ([128, 2 * HID], BF),
    "out_T": ([128, 2 * V], BF),
    "embw_pre": ([128, 2 * T], F32),
    "out_b_col": ([128, 1], F32),
    "m2t": ([128, NJ * HID], BF),
}


def build_kernel(debug=False):
    _install_shim()
    nc = bass.Bass()
    dins = {
        k: nc.dram_tensor(k, s, d, kind="ExternalInput")
        for k, (s, d) in _INPUT_SPEC.items()
    }
    out_ext = nc.dram_tensor("out", [T, V], F32, kind="ExternalOutput")
    p2d = nc.dram_tensor("p2d", [P2D_LEN], BF)
    dbg = None
    if debug:
        dbg = {
            "dbg_e8": nc.dram_tensor("dbg_e8", [128, NJ * T], BF, kind="ExternalOutput"),
            "dbg_h": nc.dram_tensor("dbg_h", [128, 2 * (T + 1)], BF, kind="ExternalOutput"),
            "dbg_rec": nc.dram_tensor("dbg_rec", [128, T], F32, kind="ExternalOutput"),
        }
    with tile.TileContext(nc) as tc:
        _build_body(nc, tc, dins, out_ext, p2d, dbg)
    return nc


def _build_body(nc, tc, dins, out_ext, p2d, dbg=None):
    AF = mybir.ActivationFunctionType

    with (
        tc.tile_pool(name="const", bufs=1) as cpool,
        tc.tile_pool(name="state", bufs=1) as spool,
        tc.tile_pool(name="p2rep", bufs=2) as rpool,
        tc.tile_pool(name="score", bufs=3) as scpool,
        tc.tile_pool(name="small", bufs=3) as smpool,
        tc.tile_pool(name="ps_big", bufs=2, space="PSUM") as ps_big,
        tc.tile_pool(name="ps_small", bufs=4, space="PSUM") as ps_small,
    ):
        big = lambda: ps_big.tile([128, HW], F32, tag="big", name="bigps")
        sm = lambda p_, f_: ps_small.tile([p_, f_], F32, tag="sm", name="smps")

        # ---- load all inputs to SBUF
        sb = {}
        for k, hndl in dins.items():
            t = cpool.tile(list(hndl.shape), hndl.dtype, tag=k)
            nc.sync.dma_start(t[:], hndl[:])
            sb[k] = t

        ident = cpool.tile([128, 128], F32, tag="ident")
        make_identity(nc, ident[:])
        ident_bf = cpool.tile([128, 128], BF, tag="ident_bf")
        nc.vector.tensor_copy(ident_bf[:], ident[:])
        ones128_f32 = cpool.tile([128, 128], F32, tag="ones128")
        nc.gpsimd.memset(ones128_f32[:], 1.0)

        # zero padded alpha staging buffer in DRAM
        zrow = cpool.tile([1, P2D_LEN], BF, tag="zrow")
        nc.gpsimd.memset(zrow[:], 0.0)
        nc.sync.dma_start(bass.AP(p2d, 0, [[P2D_LEN, 1], [1, P2D_LEN]]), zrow[:])

        # ---- persistent state / histories
        hidden = spool.tile([128, 2], F32, tag="hidden")
        alpha_bf = spool.tile([128, NJ], BF, tag="alpha_bf")
        e8_hist = spool.tile([128, NJ * T], BF, tag="e8_hist")
        hbf_hist = spool.tile([128, 2 * (T + 1)], BF, tag="hbf_hist")
        rec_hist = spool.tile([128, T], F32, tag="rec_hist")
        nc.gpsimd.memset(alpha_bf[:], 0.0)
        nc.vector.tensor_copy(hidden[:], sb["h0"][:])
        nc.vector.tensor_copy(hbf_hist[:, 0:2], sb["h0"][:])

        gi_view = sb["gi_all"][:].rearrange("p (m t) -> p t m", t=T)
        e8v = e8_hist[:].rearrange("p (t j) -> p j t", j=NJ)
        hbv = hbf_hist[:].rearrange("p (t k) -> p k t", k=2)

        # =================================================== decode loop
        p2rep = None
        for t in range(T):
            # ---- GRU (reads hbf_hist slice t = h(t-1))
            gh_ps = sm(128, 6)
            for mc in range(6):
                for kc in range(2):
                    nc.tensor.matmul(
                        gh_ps[:, mc : mc + 1],
                        sb["w_hhT"][:, kc * 768 + mc * 128 : kc * 768 + (mc + 1) * 128],
                        hbf_hist[:, 2 * t + kc : 2 * t + kc + 1],
                        start=(kc == 0 and mc == 0),
                        stop=(kc == 1 and mc == 5),
                        skip_group_check=True,
                    )
            rz_pre = smpool.tile([128, 4], F32, tag="rzpre")
            nc.vector.tensor_add(rz_pre[:], gh_ps[:, 0:4], gi_view[:, t, 0:4])
            rz_th = smpool.tile([128, 4], F32, tag="rzth")
            nc.scalar.activation(rz_th[:], rz_pre[:], AF.Tanh, scale=0.5)
            rz_sig = smpool.tile([128, 4], F32, tag="rzsig")
            nc.vector.tensor_scalar(
                rz_sig[:], rz_th[:], 0.5, 0.5,
                op0=mybir.AluOpType.mult, op1=mybir.AluOpType.add,
            )
            ghn_b = smpool.tile([128, 2], F32, tag="ghnb")
            nc.vector.tensor_add(ghn_b[:], gh_ps[:, 4:6], sb["bhn_col"][:])
            n_pre = smpool.tile([128, 2], F32, tag="npre")
            nc.vector.tensor_mul(n_pre[:], rz_sig[:, 0:2], ghn_b[:])
            n_pre2 = smpool.tile([128, 2], F32, tag="npre2")
            nc.vector.tensor_add(n_pre2[:], n_pre[:], gi_view[:, t, 4:6])
            n_sb = smpool.tile([128, 2], F32, tag="nsb")
            nc.scalar.activation(n_sb[:], n_pre2[:], AF.Tanh)
            hmn = smpool.tile([128, 2], F32, tag="hmn")
            nc.vector.tensor_sub(hmn[:], hidden[:], n_sb[:])
            zhm = smpool.tile([128, 2], F32, tag="zhm")
            nc.vector.tensor_mul(zhm[:], rz_sig[:, 2:4], hmn[:])
            nc.vector.tensor_add(hidden[:], n_sb[:], zhm[:])
            nc.vector.tensor_copy(hbf_hist[:, 2 * (t + 1) : 2 * (t + 2)], hidden[:])

            # ---- query [128, ND] (d on partitions within chunk, col = dc)
            q_ps = sm(128, ND)
            for mc in range(ND):
                for kc in range(2):
                    nc.tensor.matmul(
                        q_ps[:, mc : mc + 1],
                        sb["att_hT"][:, kc * AD + mc * 128 : kc * AD + (mc + 1) * 128],
                        hbf_hist[:, 2 * (t + 1) + kc : 2 * (t + 1) + kc + 1],
                        start=(kc == 0 and mc == 0),
                        stop=(kc == 1 and mc == ND - 1),
                        skip_group_check=True,
                    )
            query_sb = smpool.tile([128, ND], F32, tag="query")
            nc.vector.tensor_add(query_sb[:], q_ps[:], sb["att_hb_col"][:])

            # ---- coverage conv + trans -> tanh -> energy
            # lnmask term first: independent of the gather, runs early
            energy_ps = sm(128, NJ)
            nc.tensor.matmul(
                energy_ps[:],
                ident_bf[:],
                sb["lnmask_col"][:],
                start=True,
                stop=False,
                skip_group_check=True,
            )
            for dc in range(ND):
                cov_ps = big()
                if t > 0:
                    for hf in range(2):
                        nc.tensor.matmul(
                            cov_ps[:, hf * 512 : (hf + 1) * 512],
                            sb["k2"][:, dc * 128 : (dc + 1) * 128],
                            p2rep_v[:, hf * 8 : (hf + 1) * 8, 0:64],
                            start=True,
                            stop=False,
                            skip_group_check=True,
                        )
                for hf in range(2):
                    o0 = dc * HW + hf * 512
                    nc.tensor.matmul(
                        cov_ps[:, hf * 512 : (hf + 1) * 512],
                        ident_bf[:],
                        sb["trans"][:, o0 : o0 + 512],
                        start=(t == 0),
                        stop=True,
                        skip_group_check=True,
                    )
                sc = scpool.tile([128, HW], BF, tag="sc")
                nc.scalar.activation(
                    sc[:], cov_ps[:], AF.Tanh, bias=query_sb[:, dc : dc + 1]
                )
                for j in range(NJ):
                    nc.tensor.matmul(
                        energy_ps[:, j : j + 1],
                        sc[:, j * 128 : (j + 1) * 128],
                        sb["w_col"][:, dc : dc + 1],
                        start=False,
                        stop=(dc == ND - 1 and j == NJ - 1),
                        skip_group_check=True,
                    )

            # ---- softmax pieces (no max subtraction)
            esum = smpool.tile([128, 1], F32, tag="esum")
            nc.scalar.activation(
                e8_hist[:, NJ * t : NJ * (t + 1)],
                energy_ps[:],
                AF.Exp,
                bias=sb["ab_col"][:, 0:1],
                accum_out=esum[:],
            )
            sb_ps = sm(128, 1)
            nc.tensor.matmul(sb_ps[:], ones128_f32[:], esum[:], start=True, stop=True)
            nc.vector.reciprocal(rec_hist[:, t : t + 1], sb_ps[:])
            nc.vector.scalar_tensor_tensor(
                alpha_bf[:],
                e8_hist[:, NJ * t : NJ * (t + 1)],
                rec_hist[:, t : t + 1],
                alpha_bf[:],
                op0=mybir.AluOpType.mult,
                op1=mybir.AluOpType.add,
            )

            # scatter updated alpha_sum into padded DRAM; gather next im2col
            if t < T - 1:
                for q in range(2):
                    nc.sync.dma_start(
                        bass.AP(p2d, (5 + q) * PSTR + 5, [[1, 64], [2 * PSTR, NJ]]),
                        alpha_bf[64 * q : 64 * q + 64, :],
                    )
                p2rep = rpool.tile([121, GLEN], BF, tag="p2rep")
                nc.sync.dma_start(
                    p2rep[:],
                    bass.AP(p2d, 0, [[PSTR, 11], [1, 11], [1, GLEN]]),
                )
                p2rep_v = p2rep[:].rearrange("k (h w) -> k h w", w=PSTR)

        # =================================================== batched tail
        os_bf = []
        for mc in range(2):
            ctx_ps = sm(128, T)
            for j in range(NJ):
                nc.tensor.matmul(
                    ctx_ps[:],
                    sb["m2t"][:, j * HID + mc * 128 : j * HID + (mc + 1) * 128],
                    e8v[:, j, :],
                    start=(j == 0),
                    stop=(j == NJ - 1),
                    skip_group_check=True,
                )
            os_ps = sm(128, T)
            for kc in range(2):
                nc.tensor.matmul(
                    os_ps[:],
                    sb["state_T"][:, kc * HID + mc * 128 : kc * HID + (mc + 1) * 128],
                    hbv[:, kc, 1 : T + 1],
                    start=(kc == 0),
                    stop=(kc == 1),
                    skip_group_check=True,
                )
            t1 = smpool.tile([128, T], F32, tag="tailt1")
            nc.vector.tensor_mul(t1[:], ctx_ps[:], rec_hist[:])
            t2 = smpool.tile([128, T], F32, tag="tailt2")
            nc.vector.tensor_add(t2[:], t1[:], sb["embw_pre"][:, mc * T : (mc + 1) * T])
            ob = smpool.tile([128, T], BF, tag="tailob")
            nc.vector.tensor_add(ob[:], t2[:], os_ps[:])
            os_bf.append(ob)

        pr_ps = sm(V, T)
        for kc in range(2):
            nc.tensor.matmul(
                pr_ps[:],
                sb["out_T"][:, kc * V : (kc + 1) * V],
                os_bf[kc][:],
                start=(kc == 0),
                stop=(kc == 1),
            )
        probs_sb = smpool.tile([V, T], F32, tag="probs")
        nc.vector.tensor_scalar_add(probs_sb[:], pr_ps[:], sb["out_b_col"][0:V, 0:1])

        # =================================================== epilogue
        pt_ps = sm(T, V)
        nc.tensor.transpose(pt_ps[:], probs_sb[:], ident[0:V, 0:V])
        out_sb = smpool.tile([T, V], F32, tag="outsb")
        nc.vector.tensor_copy(out_sb[:], pt_ps[:])
        nc.sync.dma_start(out_ext[:], out_sb[:])
        if dbg is not None:
            nc.sync.dma_start(dbg["dbg_e8"][:], e8_hist[:])
            nc.sync.dma_start(dbg["dbg_h"][:], hbf_hist[:])
            nc.sync.dma_start(dbg["dbg_rec"][:], rec_hist[:])


# ------------------------------------------------------------- host driver
def _prep_core_inputs(b, d, pos_all):
    g = lambda k: np.asarray(d[k], np.float32)
    cnn = g("cnn_features")[b].reshape(C, HW)
    mask = g("images_mask")[b, 0, ::RATIO, ::RATIO]
    dm = mask.reshape(-1)
    trans = (g("enc_conv_w")[:, :, 0, 0] @ cnn
             + pos_all[b].reshape(AD, HW) + g("enc_conv_b")[:, None])
    m2 = cnn.T @ g("ctx_W").T  # [HW, HID]
    words = np.concatenate([[1], np.asarray(d["labels"])[b, :-1].astype(np.int64)])
    we = g("emb")[words]  # [T, INP]
    gi = (g("gru_w_ih") @ we.T
          + (g("gru_b_ih")
             + np.concatenate([g("gru_b_hh")[:512], np.zeros(256, np.float32)]))[:, None])
    counting_ctx = g("count_W") @ g("counting_preds")[b] + g("count_b")
    embw = (g("embw_W") @ we.T
            + (g("state_b") + g("embw_b") + g("ctx_b") + counting_ctx)[:, None])
    avg = (cnn * dm[None, :]).sum(1) / dm.sum()
    h0 = np.tanh(g("init_W") @ avg + g("init_b"))
    return {
        "trans": _bf(_chunk_k(trans)),
        "m2t": _bf(_chunk_k(m2)),
        "gi_all": _f32(_chunk_k(gi)),
        "embw_pre": _f32(_chunk_k(embw)),
        "h0": _f32(_chunk_k(h0[:, None])),
        "lnmask_col": _bf(np.log(np.maximum(dm, 1e-30)).reshape(NJ, 128).T),
    }


def _prep_shared_inputs(d):
    g = lambda k: np.asarray(d[k], np.float32)
    return {
        "k2": _bf(g("att_conv_w").reshape(AD, 121).T @ g("att_weight_W").T),
        "w_hhT": _bf(_chunk_k(g("gru_w_hh").T)),
        "att_hT": _bf(_chunk_k(g("att_hidden_W").T)),
        "state_T": _bf(_chunk_k(g("state_W").T)),
        "out_T": _bf(_chunk_k(g("out_W").T)),
        "w_col": _bf(g("alpha_convert_W")[0].reshape(ND, 128).T),
        "bhn_col": _f32(g("gru_b_hh")[512:].reshape(2, 128).T),
        "att_hb_col": _f32(g("att_hidden_b").reshape(ND, 128).T),
        "out_b_col": _f32(np.pad(g("out_b"), (0, 128 - V))[:, None]),
        "ab_col": _f32(np.full((128, 1), float(g("alpha_convert_b")[0]))),
    }


_cached = {}


def kernel(**inputs) -> np.ndarray:
    if "nc" not in _cached:
        _cached["nc"] = build_kernel()
    nc = _cached["nc"]

    mask_hw = np.asarray(inputs["images_mask"], np.float32)[:, 0, ::RATIO, ::RATIO]
    pos_all = _pos_embedding_sine(mask_hw)
    shared = _prep_shared_inputs(inputs)
    in_maps = []
    for b in range(B):
        m = dict(shared)
        m.update(_prep_core_inputs(b, inputs, pos_all))
        in_maps.append(m)

    res = run_bass_kernel_spmd(nc, in_maps, core_ids=list(range(8)))
    out = np.stack([res.results[i]["out"] for i in range(8)], axis=0)
    return out.astype(np.float32)


if __name__ == "__main__":
    ins = dict(np.load("/root/problem/inputs.npz"))
    got = kernel(**ins)
    exp = np.load("/root/problem/expected.npy")
    rel = np.linalg.norm(got - exp) / np.linalg.norm(exp)
    print("Relative error:", rel)


# revision 21
# speedup vs baseline: 1.9913x; 1.9913x over previous
"""Trainium2 Bass kernel for nn_AttDecoder (GRU + coverage attention decoder).

Sharding: pure data parallel — batch 8 across 8 NeuronCores (batch=1/core).

v5 structure (host-validated numerics, rel err ~2.4e-3):
  - all static preprocessing on host: trans(+att_hidden_b) = enc_conv(cnn)+pos,
    K2 = att_weight_W @ att_conv_w, M2T = cnn^T ctx_W^T, gi, embw/bias terms,
    hidden0. Device loads ~2MB.
  - [d, pos] orientation: score psum chunks are [128 d, 1024 pos]. Per chunk
    the trans identity-matmul is gather-INDEPENDENT, issued during the alpha
    DMA round-trip to keep the PE warm (HAM); the K=121 coverage matmul from
    the one-DMA im2col gather lands after. tanh on ACT with query as the
    per-partition bias; energy via 32 small matmuls with lnmask as the
    accumulation group's window-issued first term.
  - alpha_sum accumulated TRANSPOSED ([8, 128] bf16 via a PE transpose of e8)
    so the scatter is 16 contiguous 128B descriptors, and the gather is split
    across the SP and ACT DMA queues.
  - ctx/out_state/prob tail batched AFTER the decode loop (N=36 matmuls).
Layouts: pos = h*64+w (1024); within 128-slices pos = j*128 + 64q + w with
h = 2j+q; d contiguous 512.
"""

import json
import math
import sys

import numpy as np
import ml_dtypes

sys.path.insert(0, "/opt/trn_rl_repo")

import concourse.bass as bass
import concourse.mybir as mybir
import concourse.tile as tile
from concourse.bass_utils import run_bass_kernel_spmd
from concourse.masks import make_identity

B, C, H, W = 8, 684, 16, 64
HID, INP, AD, V, T = 256, 256, 512, 111, 36
RATIO = 16
HW = H * W
ND = AD // 128
NJ = HW // 128
PSTR = 84
GLEN = 16 * PSTR  # im2col gather row length (16 h-rows x 84 stride)
P2D_LEN = 3072
BF = mybir.dt.bfloat16
F32 = mybir.dt.float32

_bf = lambda x: np.ascontiguousarray(np.asarray(x, dtype=np.float32)).astype(
    ml_dtypes.bfloat16
)
_f32 = lambda x: np.ascontiguousarray(np.asarray(x, dtype=np.float32))


def _chunk_k(a, k_pad=None):
    """[K, M] -> [128, (K/128)*M]; out[p, kc*M+m] = a[kc*128+p, m]."""
    a = np.asarray(a, dtype=np.float32)
    k, m = a.shape
    kp = k_pad or k
    if kp > k:
        a = np.concatenate([a, np.zeros((kp - k, m), np.float32)], 0)
    nk = kp // 128
    assert nk * 128 == kp
    return np.ascontiguousarray(
        a.reshape(nk, 128, m).transpose(1, 0, 2).reshape(128, nk * m)
    )


def _pos_embedding_sine(mask_hw):
    """numpy port of reference.pos_embedding_sine; [B,H,W] -> [B,512,H,W]."""
    num_pos_feats, temperature = 256, 10000.0
    scale = 2.0 * math.pi
    eps = 1e-6
    m = np.asarray(mask_hw, np.float32)
    y = np.cumsum(m, axis=1)
    x = np.cumsum(m, axis=2)
    y = y / (y[:, -1:, :] + eps) * scale
    x = x / (x[:, :, -1:] + eps) * scale
    i = np.arange(num_pos_feats, dtype=np.float32)
    dim_t = temperature ** (2.0 * np.floor(i / 2.0) / num_pos_feats)
    px = x[..., None] / dim_t
    py = y[..., None] / dim_t

    def inter(p):
        return np.stack((np.sin(p[..., 0::2]), np.cos(p[..., 1::2])), axis=4).reshape(
            p.shape[:3] + (num_pos_feats,)
        )

    pos = np.concatenate((inter(py), inter(px)), axis=3)
    return np.transpose(pos, (0, 3, 1, 2))


# ------------------------------------------------- walrus wait-split shim
def _split_sync_waits(bir_json: bytes, max_waits: int = 1) -> bytes:
    """This walrus build encodes one sem wait per instruction; hoist extras
    onto NoOps inserted before the instruction on the same engine."""
    js = json.loads(bir_json)
    n = 0
    for fn in js.get("functions", []):
        for bb in fn.get("blocks", []):
            out = []
            for ins in bb.get("instructions", []):
                si = ins.get("sync_info")
                waits = (si or {}).get("on_wait") or []
                upds = (si or {}).get("on_update") or []
                assert len(upds) <= 1, ins.get("name")
                if len(waits) > max_waits:
                    extra, si["on_wait"] = waits[:-max_waits], waits[-max_waits:]
                    for w in extra:
                        n += 1
                        out.append(
                            {
                                "debug": ins.get("debug", 0),
                                "engine": ins["engine"],
                                "ins": [],
                                "outs": [],
                                "name": f"WSPLIT-{n}",
                                "opcode": "NoOp",
                                "sync_info": {"on_wait": [w], "on_update": []},
                            }
                        )
                out.append(ins)
            bb["instructions"] = out
    return json.dumps(js).encode()


_shim_installed = False


def _install_shim():
    global _shim_installed
    if _shim_installed:
        return
    import concourse.bass2jax as bass2jax

    orig = bass2jax.compile_bir_kernel

    def wrapper(bir_json, tmpdir, neff_name="file.neff"):
        return orig(_split_sync_waits(bir_json), tmpdir, neff_name)

    bass2jax.compile_bir_kernel = wrapper
    _shim_installed = True


# ------------------------------------------------------------ bass builder
_INPUT_SPEC = {
    # small, needed first in the decode loop
    "gi_all": ([128, 6 * T], F32),
    "h0": ([128, 2], F32),
    "w_hhT": ([128, 2 * 3 * HID], BF),
    "att_hT": ([128, 2 * AD], BF),
    "lnmask_col": ([128, NJ], BF),
    "w_col": ([128, ND], BF),
    "bhn_col": ([128, 2], F32),
    "att_hb_col": ([128, ND], F32),
    "ab_col": ([128, 1], F32),
    "k2": ([121, AD], BF),
    # big per-core tensors
    "trans": ([128, ND * HW], BF),
    # tail-only
    "state_T": ([128, 2 * HID], BF),
    "out_T": ([128, 2 * V], BF),
    "embw_pre": ([128, 2 * T], F32),
    "out_b_col": ([128, 1], F32),
    "m2t": ([128, NJ * HID], BF),
}


def build_kernel(debug=False):
    _install_shim()
    nc = bass.Bass()
    dins = {
        k: nc.dram_tensor(k, s, d, kind="ExternalInput")
        for k, (s, d) in _INPUT_SPEC.items()
    }
    out_ext = nc.dram_tensor("out", [T, V], F32, kind="ExternalOutput")
    p2d = nc.dram_tensor("p2d", [P2D_LEN], BF)
    dbg = None
    if debug:
        dbg = {
            "dbg_e8": nc.dram_tensor("dbg_e8", [128, NJ * T], BF, kind="ExternalOutput"),
            "dbg_h": nc.dram_tensor("dbg_h", [128, 2 * (T + 1)], BF, kind="ExternalOutput"),
            "dbg_rec": nc.dram_tensor("dbg_rec", [128, T], F32, kind="ExternalOutput"),
        }
    with tile.TileContext(nc) as tc:
        _build_body(nc, tc, dins, out_ext, p2d, dbg)
    return nc


def _build_body(nc, tc, dins, out_ext, p2d, dbg=None):
    AF = mybir.ActivationFunctionType
    ALU = mybir.AluOpType

    with (
        tc.tile_pool(name="const", bufs=1) as cpool,
        tc.tile_pool(name="state", bufs=1) as spool,
        tc.tile_pool(name="p2rep", bufs=2) as rpool,
        tc.tile_pool(name="score", bufs=2) as scpool,
        tc.tile_pool(name="small", bufs=3) as smpool,
        tc.tile_pool(name="ps_big", bufs=3, space="PSUM") as ps_big,
        tc.tile_pool(name="ps_small", bufs=2, space="PSUM") as ps_small,
    ):
        big = lambda: ps_big.tile([128, HW], F32, tag="big", name="bigps")
        sm = lambda p_, f_: ps_small.tile([p_, f_], F32, tag="sm", name="smps")

        # ---- load all inputs to SBUF
        sb = {}
        for k, hndl in dins.items():
            t = cpool.tile(list(hndl.shape), hndl.dtype, tag=k)
            nc.sync.dma_start(t[:], hndl[:])
            sb[k] = t

        ident = cpool.tile([128, 128], F32, tag="ident")
        make_identity(nc, ident[:])
        ident_bf = cpool.tile([128, 128], BF, tag="ident_bf")
        nc.vector.tensor_copy(ident_bf[:], ident[:])
        ones128_f32 = cpool.tile([128, 128], F32, tag="ones128")
        nc.gpsimd.memset(ones128_f32[:], 1.0)

        # zero padded alpha staging buffer in DRAM
        zrow = cpool.tile([1, P2D_LEN], BF, tag="zrow")
        nc.gpsimd.memset(zrow[:], 0.0)
        nc.sync.dma_start(bass.AP(p2d, 0, [[P2D_LEN, 1], [1, P2D_LEN]]), zrow[:])

        # ---- persistent state / histories
        hidden = spool.tile([128, 2], F32, tag="hidden")
        alpha_t8 = spool.tile([NJ, 128], BF, tag="alpha_t8")
        e8_hist = spool.tile([128, NJ * T], BF, tag="e8_hist")
        hbf_hist = spool.tile([128, 2 * (T + 1)], BF, tag="hbf_hist")
        rec_hist = spool.tile([128, T], F32, tag="rec_hist")
        nc.gpsimd.memset(alpha_t8[:], 0.0)
        nc.vector.tensor_copy(hidden[:], sb["h0"][:])
        nc.vector.tensor_copy(hbf_hist[:, 0:2], sb["h0"][:])

        gi_view = sb["gi_all"][:].rearrange("p (m t) -> p t m", t=T)
        e8v = e8_hist[:].rearrange("p (t j) -> p j t", j=NJ)
        hbv = hbf_hist[:].rearrange("p (t k) -> p k t", k=2)

        # =================================================== decode loop
        p2rep_v = None
        for t in range(T):
            # ---- GRU (reads hbf_hist slice t = h(t-1))
            gh_ps = sm(128, 6)
            for mc in range(6):
                for kc in range(2):
                    nc.tensor.matmul(
                        gh_ps[:, mc : mc + 1],
                        sb["w_hhT"][:, kc * 768 + mc * 128 : kc * 768 + (mc + 1) * 128],
                        hbf_hist[:, 2 * t + kc : 2 * t + kc + 1],
                        start=(kc == 0 and mc == 0),
                        stop=(kc == 1 and mc == 5),
                        skip_group_check=True,
                    )
            rz_pre = smpool.tile([128, 4], F32, tag="rzpre")
            nc.vector.tensor_add(rz_pre[:], gh_ps[:, 0:4], gi_view[:, t, 0:4])
            rz_th = smpool.tile([128, 4], F32, tag="rzth")
            nc.scalar.activation(rz_th[:], rz_pre[:], AF.Tanh, scale=0.5)
            rz_sig = smpool.tile([128, 4], F32, tag="rzsig")
            nc.vector.tensor_scalar(
                rz_sig[:], rz_th[:], 0.5, 0.5, op0=ALU.mult, op1=ALU.add,
            )
            ghn_b = smpool.tile([128, 2], F32, tag="ghnb")
            nc.vector.tensor_add(ghn_b[:], gh_ps[:, 4:6], sb["bhn_col"][:])
            n_pre = smpool.tile([128, 2], F32, tag="npre")
            nc.vector.tensor_mul(n_pre[:], rz_sig[:, 0:2], ghn_b[:])
            n_pre2 = smpool.tile([128, 2], F32, tag="npre2")
            nc.vector.tensor_add(n_pre2[:], n_pre[:], gi_view[:, t, 4:6])
            n_sb = smpool.tile([128, 2], F32, tag="nsb")
            nc.scalar.activation(n_sb[:], n_pre2[:], AF.Tanh)
            hmn = smpool.tile([128, 2], F32, tag="hmn")
            nc.vector.tensor_sub(hmn[:], hidden[:], n_sb[:])
            zhm = smpool.tile([128, 2], F32, tag="zhm")
            nc.vector.tensor_mul(zhm[:], rz_sig[:, 2:4], hmn[:])
            nc.vector.tensor_add(hidden[:], n_sb[:], zhm[:])
            nc.vector.tensor_copy(hbf_hist[:, 2 * (t + 1) : 2 * (t + 2)], hidden[:])

            # ---- query [128, ND] (d on partitions within chunk, col = dc)
            q_ps = sm(128, ND)
            for mc in range(ND):
                for kc in range(2):
                    nc.tensor.matmul(
                        q_ps[:, mc : mc + 1],
                        sb["att_hT"][:, kc * AD + mc * 128 : kc * AD + (mc + 1) * 128],
                        hbf_hist[:, 2 * (t + 1) + kc : 2 * (t + 1) + kc + 1],
                        start=(kc == 0 and mc == 0),
                        stop=(kc == 1 and mc == ND - 1),
                        skip_group_check=True,
                    )
            query_sb = smpool.tile([128, ND], F32, tag="query")
            nc.vector.tensor_add(query_sb[:], q_ps[:], sb["att_hb_col"][:])

            # ---- per-d-chunk psum groups [128 d, 1024 pos]: trans identity
            # is gather-INDEPENDENT, issued during the alpha DMA round-trip to
            # keep the PE warm; the K=121 coverage matmul lands after the
            # gather. Chunks 0-2 pre-issued (3 psum ring slots); chunk 3's
            # group is emitted after tanh(0) so its WAR on the recycled slot
            # cannot block conv(0). The lnmask energy term is window work too.
            energy_sb = sm(128, NJ)
            nc.tensor.matmul(
                energy_sb[:], ident_bf[:], sb["lnmask_col"][:],
                start=True, stop=False, skip_group_check=True,
            )

            def trans_mms(dc):
                cov_ps = big()
                for hf in range(2):
                    o0 = dc * HW + hf * 512
                    nc.tensor.matmul(
                        cov_ps[:, hf * 512 : hf * 512 + 512],
                        ident_bf[:],
                        sb["trans"][:, o0 : o0 + 512],
                        start=True,
                        stop=(t == 0),
                        skip_group_check=True,
                    )
                return cov_ps

            def convs(dc, cov_ps):
                for hf in range(2):
                    nc.tensor.matmul(
                        cov_ps[:, hf * 512 : hf * 512 + 512],
                        sb["k2"][:, dc * 128 : (dc + 1) * 128],
                        p2rep_v[:, hf * 8 : (hf + 1) * 8, 0:64],
                        start=False,
                        stop=True,
                        skip_group_check=True,
                    )

            def tanh_energy(dc, cov_ps):
                sc = scpool.tile([128, HW], BF, tag="sc")
                nc.scalar.activation(
                    sc[:], cov_ps[:], AF.Tanh, bias=query_sb[:, dc : dc + 1]
                )
                for j in range(NJ):
                    nc.tensor.matmul(
                        energy_sb[:, j : j + 1],
                        sc[:, j * 128 : (j + 1) * 128],
                        sb["w_col"][:, dc : dc + 1],
                        start=False,
                        stop=(dc == ND - 1 and j == NJ - 1),
                        skip_group_check=True,
                    )

            cov_list = [trans_mms(dc) for dc in range(3)]
            if t > 0:
                for dc in range(3):
                    convs(dc, cov_list[dc])
            tanh_energy(0, cov_list[0])
            cov3 = trans_mms(3)
            if t > 0:
                convs(3, cov3)
            for dc in range(1, 3):
                tanh_energy(dc, cov_list[dc])
            tanh_energy(3, cov3)

            # ---- softmax pieces (no max subtraction)
            esum = smpool.tile([128, 1], F32, tag="esum")
            nc.scalar.activation(
                e8_hist[:, NJ * t : NJ * (t + 1)],
                energy_sb[:],
                AF.Exp,
                bias=sb["ab_col"][:, 0:1],
                accum_out=esum[:],
            )
            sb_ps = sm(128, 1)
            nc.tensor.matmul(sb_ps[:], ones128_f32[:], esum[:], start=True, stop=True)
            nc.vector.reciprocal(rec_hist[:, t : t + 1], sb_ps[:])

            # alpha_sum is accumulated TRANSPOSED ([8 j, 128 (q,w)]) so the
            # scatter below is 16 contiguous 128B descriptors instead of a
            # partition-transposing (2-bytes-per-descriptor) write.
            if t < T - 1:
                e8t_ps = ps_small.tile([NJ, 128], BF, tag="sm", name="smps")
                nc.tensor.transpose(
                    e8t_ps[:], e8_hist[:, NJ * t : NJ * (t + 1)], ident_bf[:]
                )
                nc.vector.scalar_tensor_tensor(
                    alpha_t8[:],
                    e8t_ps[:],
                    rec_hist[0:NJ, t : t + 1],
                    alpha_t8[:],
                    op0=ALU.mult,
                    op1=ALU.add,
                )
                nc.sync.dma_start(
                    bass.AP(p2d, 5 * PSTR + 5, [[2 * PSTR, NJ], [PSTR, 2], [1, 64]]),
                    alpha_t8[:],
                )
                p2rep = rpool.tile([121, GLEN], BF, tag="p2rep")
                nc.sync.dma_start(
                    p2rep[0:66, :],
                    bass.AP(p2d, 0, [[PSTR, 6], [1, 11], [1, GLEN]]),
                )
                nc.scalar.dma_start(
                    p2rep[66:121, :],
                    bass.AP(p2d, 6 * PSTR, [[PSTR, 5], [1, 11], [1, GLEN]]),
                )
                p2rep_v = p2rep[:].rearrange("k (h w) -> k h w", w=PSTR)

        # =================================================== batched tail
        os_bf = []
        for mc in range(2):
            ctx_ps = sm(128, T)
            for j in range(NJ):
                nc.tensor.matmul(
                    ctx_ps[:],
                    sb["m2t"][:, j * HID + mc * 128 : j * HID + (mc + 1) * 128],
                    e8v[:, j, :],
                    start=(j == 0),
                    stop=(j == NJ - 1),
                    skip_group_check=True,
                )
            os_ps = sm(128, T)
            for kc in range(2):
                nc.tensor.matmul(
                    os_ps[:],
                    sb["state_T"][:, kc * HID + mc * 128 : kc * HID + (mc + 1) * 128],
                    hbv[:, kc, 1 : T + 1],
                    start=(kc == 0),
                    stop=(kc == 1),
                    skip_group_check=True,
                )
            t1 = smpool.tile([128, T], F32, tag="tailt1")
            nc.vector.tensor_mul(t1[:], ctx_ps[:], rec_hist[:])
            t2 = smpool.tile([128, T], F32, tag="tailt2")
            nc.vector.tensor_add(t2[:], t1[:], sb["embw_pre"][:, mc * T : (mc + 1) * T])
            ob = smpool.tile([128, T], BF, tag="tailob")
            nc.vector.tensor_add(ob[:], t2[:], os_ps[:])
            os_bf.append(ob)

        pr_ps = sm(V, T)
        for kc in range(2):
            nc.tensor.matmul(
                pr_ps[:],
                sb["out_T"][:, kc * V : (kc + 1) * V],
                os_bf[kc][:],
                start=(kc == 0),
                stop=(kc == 1),
            )
        probs_sb = smpool.tile([V, T], F32, tag="probs")
        nc.vector.tensor_scalar_add(probs_sb[:], pr_ps[:], sb["out_b_col"][0:V, 0:1])

        # =================================================== epilogue
        pt_ps = sm(T, V)
        nc.tensor.transpose(pt_ps[:], probs_sb[:], ident[0:V, 0:V])
        out_sb = smpool.tile([T, V], F32, tag="outsb")
        nc.vector.tensor_copy(out_sb[:], pt_ps[:])
        nc.sync.dma_start(out_ext[:], out_sb[:])
        if dbg is not None:
            nc.sync.dma_start(dbg["dbg_e8"][:], e8_hist[:])
            nc.sync.dma_start(dbg["dbg_h"][:], hbf_hist[:])
            nc.sync.dma_start(dbg["dbg_rec"][:], rec_hist[:])


# ------------------------------------------------------------- host driver
def _prep_core_inputs(b, d, pos_all):
    g = lambda k: np.asarray(d[k], np.float32)
    cnn = g("cnn_features")[b].reshape(C, HW)
    mask = g("images_mask")[b, 0, ::RATIO, ::RATIO]
    dm = mask.reshape(-1)
    trans = (g("enc_conv_w")[:, :, 0, 0] @ cnn
             + pos_all[b].reshape(AD, HW) + g("enc_conv_b")[:, None])
    m2 = cnn.T @ g("ctx_W").T  # [HW, HID]
    words = np.concatenate([[1], np.asarray(d["labels"])[b, :-1].astype(np.int64)])
    we = g("emb")[words]  # [T, INP]
    gi = (g("gru_w_ih") @ we.T
          + (g("gru_b_ih")
             + np.concatenate([g("gru_b_hh")[:512], np.zeros(256, np.float32)]))[:, None])
    counting_ctx = g("count_W") @ g("counting_preds")[b] + g("count_b")
    embw = (g("embw_W") @ we.T
            + (g("state_b") + g("embw_b") + g("ctx_b") + counting_ctx)[:, None])
    avg = (cnn * dm[None, :]).sum(1) / dm.sum()
    h0 = np.tanh(g("init_W") @ avg + g("init_b"))
    return {
        "trans": _bf(_chunk_k(trans)),
        "m2t": _bf(_chunk_k(m2)),
        "gi_all": _f32(_chunk_k(gi)),
        "embw_pre": _f32(_chunk_k(embw)),
        "h0": _f32(_chunk_k(h0[:, None])),
        "lnmask_col": _f32(np.log(np.maximum(dm, 1e-30)).reshape(NJ, 128).T),
    }


def _prep_shared_inputs(d):
    g = lambda k: np.asarray(d[k], np.float32)
    return {
        "k2": _bf(g("att_conv_w").reshape(AD, 121).T @ g("att_weight_W").T),
        "w_hhT": _bf(_chunk_k(g("gru_w_hh").T)),
        "att_hT": _bf(_chunk_k(g("att_hidden_W").T)),
        "state_T": _bf(_chunk_k(g("state_W").T)),
        "out_T": _bf(_chunk_k(g("out_W").T)),
        "w_col": _bf(g("alpha_convert_W")[0].reshape(ND, 128).T),
        "att_hb_col": _f32(g("att_hidden_b").reshape(ND, 128).T),
        "bhn_col": _f32(g("gru_b_hh")[512:].reshape(2, 128).T),
        "out_b_col": _f32(np.pad(g("out_b"), (0, 128 - V))[:, None]),
        "ab_col": _f32(np.full((128, 1), float(g("alpha_convert_b")[0]))),
    }


_cached = {}


def kernel(**inputs) -> np.ndarray:
    if "nc" not in _cached:
        _cached["nc"] = build_kernel()
    nc = _cached["nc"]

    mask_hw = np.asarray(inputs["images_mask"], np.float32)[:, 0, ::RATIO, ::RATIO]
    pos_all = _pos_embedding_sine(mask_hw)
    shared = _prep_shared_inputs(inputs)
    in_maps = []
    for b in range(B):
        m = dict(shared)
        m.update(_prep_core_inputs(b, inputs, pos_all))
        in_maps.append(m)

    res = run_bass_kernel_spmd(nc, in_maps, core_ids=list(range(8)))
    out = np.stack([res.results[i]["out"] for i in range(8)], axis=0)
    return out.astype(np.float32)


if __name__ == "__main__":
    ins = dict(np.load("/root/problem/inputs.npz"))
    got = kernel(**ins)
    exp = np.load("/root/problem/expected.npy")
    rel = np.linalg.norm(got - exp) / np.linalg.norm(exp)
    print("Relative error:", rel)


# revision 24
# speedup vs baseline: 2.1644x; 1.0870x over previous
"""Trainium2 Bass kernel for nn_AttDecoder (GRU + coverage attention decoder).

Sharding: pure data parallel — batch 8 across 8 NeuronCores (batch=1/core).

v5 structure (host-validated numerics, rel err ~2.4e-3):
  - all static preprocessing on host: trans(+att_hidden_b) = enc_conv(cnn)+pos,
    K2 = att_weight_W @ att_conv_w, M2T = cnn^T ctx_W^T, gi, embw/bias terms,
    hidden0. Device loads ~2MB.
  - [d, pos] orientation: score psum chunks are [128 d, 1024 pos]. Per chunk
    the trans identity-matmul is gather-INDEPENDENT, issued during the alpha
    DMA round-trip to keep the PE warm (HAM); the K=121 coverage matmul from
    the one-DMA im2col gather lands after. tanh on ACT with query as the
    per-partition bias; energy via 32 small matmuls with lnmask as the
    accumulation group's window-issued first term.
  - alpha_sum accumulated TRANSPOSED ([8, 128] bf16 via a PE transpose of e8)
    so the scatter is 16 contiguous 128B descriptors, and the gather is split
    across the SP and ACT DMA queues.
  - ctx/out_state/prob tail batched AFTER the decode loop (N=36 matmuls).
Layouts: pos = h*64+w (1024); within 128-slices pos = j*128 + 64q + w with
h = 2j+q; d contiguous 512.
"""

import json
import math
import sys

import numpy as np
import ml_dtypes

sys.path.insert(0, "/opt/trn_rl_repo")

import concourse.bass as bass
import concourse.mybir as mybir
import concourse.tile as tile
from concourse.bass_utils import run_bass_kernel_spmd
from concourse.masks import make_identity

B, C, H, W = 8, 684, 16, 64
HID, INP, AD, V, T = 256, 256, 512, 111, 36
RATIO = 16
HW = H * W
ND = AD // 128
NJ = HW // 128
PSTR = 84
GLEN = 16 * PSTR  # im2col gather row length (16 h-rows x 84 stride)
P2D_LEN = 3072
BF = mybir.dt.bfloat16
F32 = mybir.dt.float32
F8 = mybir.dt.float8e4

_bf = lambda x: np.ascontiguousarray(np.asarray(x, dtype=np.float32)).astype(
    ml_dtypes.bfloat16
)
_f8 = lambda x: np.ascontiguousarray(np.asarray(x, dtype=np.float32)).astype(
    ml_dtypes.float8_e4m3
)
_f32 = lambda x: np.ascontiguousarray(np.asarray(x, dtype=np.float32))


def _chunk_k(a, k_pad=None):
    """[K, M] -> [128, (K/128)*M]; out[p, kc*M+m] = a[kc*128+p, m]."""
    a = np.asarray(a, dtype=np.float32)
    k, m = a.shape
    kp = k_pad or k
    if kp > k:
        a = np.concatenate([a, np.zeros((kp - k, m), np.float32)], 0)
    nk = kp // 128
    assert nk * 128 == kp
    return np.ascontiguousarray(
        a.reshape(nk, 128, m).transpose(1, 0, 2).reshape(128, nk * m)
    )


def _pos_embedding_sine(mask_hw):
    """numpy port of reference.pos_embedding_sine; [B,H,W] -> [B,512,H,W]."""
    num_pos_feats, temperature = 256, 10000.0
    scale = 2.0 * math.pi
    eps = 1e-6
    m = np.asarray(mask_hw, np.float32)
    y = np.cumsum(m, axis=1)
    x = np.cumsum(m, axis=2)
    y = y / (y[:, -1:, :] + eps) * scale
    x = x / (x[:, :, -1:] + eps) * scale
    i = np.arange(num_pos_feats, dtype=np.float32)
    dim_t = temperature ** (2.0 * np.floor(i / 2.0) / num_pos_feats)
    px = x[..., None] / dim_t
    py = y[..., None] / dim_t

    def inter(p):
        return np.stack((np.sin(p[..., 0::2]), np.cos(p[..., 1::2])), axis=4).reshape(
            p.shape[:3] + (num_pos_feats,)
        )

    pos = np.concatenate((inter(py), inter(px)), axis=3)
    return np.transpose(pos, (0, 3, 1, 2))


# ------------------------------------------------- walrus wait-split shim
def _split_sync_waits(bir_json: bytes, max_waits: int = 1) -> bytes:
    """This walrus build encodes one sem wait per instruction; hoist extras
    onto NoOps inserted before the instruction on the same engine."""
    js = json.loads(bir_json)
    n = 0
    for fn in js.get("functions", []):
        for bb in fn.get("blocks", []):
            out = []
            for ins in bb.get("instructions", []):
                si = ins.get("sync_info")
                waits = (si or {}).get("on_wait") or []
                upds = (si or {}).get("on_update") or []
                assert len(upds) <= 1, ins.get("name")
                if len(waits) > max_waits:
                    extra, si["on_wait"] = waits[:-max_waits], waits[-max_waits:]
                    for w in extra:
                        n += 1
                        out.append(
                            {
                                "debug": ins.get("debug", 0),
                                "engine": ins["engine"],
                                "ins": [],
                                "outs": [],
                                "name": f"WSPLIT-{n}",
                                "opcode": "NoOp",
                                "sync_info": {"on_wait": [w], "on_update": []},
                            }
                        )
                out.append(ins)
            bb["instructions"] = out
    return json.dumps(js).encode()


_shim_installed = False


def _install_shim():
    global _shim_installed
    if _shim_installed:
        return
    import concourse.bass2jax as bass2jax

    orig = bass2jax.compile_bir_kernel

    def wrapper(bir_json, tmpdir, neff_name="file.neff"):
        return orig(_split_sync_waits(bir_json), tmpdir, neff_name)

    bass2jax.compile_bir_kernel = wrapper
    _shim_installed = True


# ------------------------------------------------------------ bass builder
_INPUT_SPEC = {
    # small, needed first in the decode loop
    "gi_all": ([128, 6 * T], F32),
    "h0": ([128, 2], F32),
    "w_hhT": ([128, 2 * 3 * HID], BF),
    "att_hT": ([128, 2 * AD], BF),
    "lnmask_col": ([128, NJ], BF),
    "w_col": ([128, ND], BF),
    "bhn_col": ([128, 2], F32),
    "att_hb_col": ([128, ND], F32),
    "ab_col": ([128, 1], F32),
    "k2": ([121, AD], F8),
    # big per-core tensors
    "trans": ([128, ND * HW], BF),
    # tail-only
    "state_T": ([128, 2 * HID], BF),
    "out_T": ([128, 2 * V], BF),
    "embw_pre": ([128, 2 * T], F32),
    "out_b_col": ([128, 1], F32),
    "m2t": ([128, NJ * HID], BF),
}


def build_kernel(debug=False):
    _install_shim()
    nc = bass.Bass()
    dins = {
        k: nc.dram_tensor(k, s, d, kind="ExternalInput")
        for k, (s, d) in _INPUT_SPEC.items()
    }
    out_ext = nc.dram_tensor("out", [T, V], F32, kind="ExternalOutput")
    p2d = nc.dram_tensor("p2d", [P2D_LEN], F8)
    dbg = None
    if debug:
        dbg = {
            "dbg_e8": nc.dram_tensor("dbg_e8", [128, NJ * T], BF, kind="ExternalOutput"),
            "dbg_h": nc.dram_tensor("dbg_h", [128, 2 * (T + 1)], BF, kind="ExternalOutput"),
            "dbg_rec": nc.dram_tensor("dbg_rec", [128, T], F32, kind="ExternalOutput"),
        }
    with tile.TileContext(nc) as tc:
        _build_body(nc, tc, dins, out_ext, p2d, dbg)
    return nc


def _build_body(nc, tc, dins, out_ext, p2d, dbg=None):
    AF = mybir.ActivationFunctionType
    ALU = mybir.AluOpType

    with (
        tc.tile_pool(name="const", bufs=1) as cpool,
        tc.tile_pool(name="state", bufs=1) as spool,
        tc.tile_pool(name="p2rep", bufs=2) as rpool,
        tc.tile_pool(name="score", bufs=2) as scpool,
        tc.tile_pool(name="small", bufs=3) as smpool,
        tc.tile_pool(name="ps_big", bufs=3, space="PSUM") as ps_big,
        tc.tile_pool(name="ps_small", bufs=2, space="PSUM") as ps_small,
    ):
        big = lambda: ps_big.tile([128, HW], F32, tag="big", name="bigps")
        sm = lambda p_, f_: ps_small.tile([p_, f_], F32, tag="sm", name="smps")

        # ---- load all inputs to SBUF
        sb = {}
        for k, hndl in dins.items():
            t = cpool.tile(list(hndl.shape), hndl.dtype, tag=k)
            nc.sync.dma_start(t[:], hndl[:])
            sb[k] = t

        ident = cpool.tile([128, 128], F32, tag="ident")
        make_identity(nc, ident[:])
        ident_bf = cpool.tile([128, 128], BF, tag="ident_bf")
        nc.vector.tensor_copy(ident_bf[:], ident[:])
        ones128_f32 = cpool.tile([128, 128], F32, tag="ones128")
        nc.gpsimd.memset(ones128_f32[:], 1.0)

        # zero padded alpha staging buffer in DRAM
        zrow = cpool.tile([1, P2D_LEN], F8, tag="zrow")
        nc.gpsimd.memset(zrow[:], 0.0)
        nc.sync.dma_start(bass.AP(p2d, 0, [[P2D_LEN, 1], [1, P2D_LEN]]), zrow[:])

        # ---- persistent state / histories
        hidden = spool.tile([128, 2], F32, tag="hidden")
        alpha_t8 = spool.tile([NJ, 128], BF, tag="alpha_t8")
        alpha_f8 = spool.tile([NJ, 128], F8, tag="alpha_f8")
        e8_hist = spool.tile([128, NJ * T], BF, tag="e8_hist")
        hbf_hist = spool.tile([128, 2 * (T + 1)], BF, tag="hbf_hist")
        rec_hist = spool.tile([128, T], F32, tag="rec_hist")
        nc.gpsimd.memset(alpha_t8[:], 0.0)
        nc.vector.tensor_copy(hidden[:], sb["h0"][:])
        nc.vector.tensor_copy(hbf_hist[:, 0:2], sb["h0"][:])

        gi_view = sb["gi_all"][:].rearrange("p (m t) -> p t m", t=T)
        e8v = e8_hist[:].rearrange("p (t j) -> p j t", j=NJ)
        hbv = hbf_hist[:].rearrange("p (t k) -> p k t", k=2)

        # =================================================== decode loop
        p2rep_v = None
        for t in range(T):
            # ---- GRU (reads hbf_hist slice t = h(t-1))
            gh_ps = sm(128, 6)
            for mc in range(6):
                for kc in range(2):
                    nc.tensor.matmul(
                        gh_ps[:, mc : mc + 1],
                        sb["w_hhT"][:, kc * 768 + mc * 128 : kc * 768 + (mc + 1) * 128],
                        hbf_hist[:, 2 * t + kc : 2 * t + kc + 1],
                        start=(kc == 0 and mc == 0),
                        stop=(kc == 1 and mc == 5),
                        skip_group_check=True,
                    )
            rz_pre = smpool.tile([128, 4], F32, tag="rzpre")
            nc.vector.tensor_add(rz_pre[:], gh_ps[:, 0:4], gi_view[:, t, 0:4])
            rz_th = smpool.tile([128, 4], F32, tag="rzth")
            nc.scalar.activation(rz_th[:], rz_pre[:], AF.Tanh, scale=0.5)
            rz_sig = smpool.tile([128, 4], F32, tag="rzsig")
            nc.vector.tensor_scalar(
                rz_sig[:], rz_th[:], 0.5, 0.5, op0=ALU.mult, op1=ALU.add,
            )
            ghn_b = smpool.tile([128, 2], F32, tag="ghnb")
            nc.vector.tensor_add(ghn_b[:], gh_ps[:, 4:6], sb["bhn_col"][:])
            n_pre = smpool.tile([128, 2], F32, tag="npre")
            nc.vector.tensor_mul(n_pre[:], rz_sig[:, 0:2], ghn_b[:])
            n_pre2 = smpool.tile([128, 2], F32, tag="npre2")
            nc.vector.tensor_add(n_pre2[:], n_pre[:], gi_view[:, t, 4:6])
            n_sb = smpool.tile([128, 2], F32, tag="nsb")
            nc.scalar.activation(n_sb[:], n_pre2[:], AF.Tanh)
            hmn = smpool.tile([128, 2], F32, tag="hmn")
            nc.vector.tensor_sub(hmn[:], hidden[:], n_sb[:])
            zhm = smpool.tile([128, 2], F32, tag="zhm")
            nc.vector.tensor_mul(zhm[:], rz_sig[:, 2:4], hmn[:])
            nc.vector.tensor_add(hidden[:], n_sb[:], zhm[:])
            nc.vector.tensor_copy(hbf_hist[:, 2 * (t + 1) : 2 * (t + 2)], hidden[:])

            # ---- query [128, ND] (d on partitions within chunk, col = dc)
            q_ps = sm(128, ND)
            for mc in range(ND):
                for kc in range(2):
                    nc.tensor.matmul(
                        q_ps[:, mc : mc + 1],
                        sb["att_hT"][:, kc * AD + mc * 128 : kc * AD + (mc + 1) * 128],
                        hbf_hist[:, 2 * (t + 1) + kc : 2 * (t + 1) + kc + 1],
                        start=(kc == 0 and mc == 0),
                        stop=(kc == 1 and mc == ND - 1),
                        skip_group_check=True,
                    )
            query_sb = smpool.tile([128, ND], F32, tag="query")
            nc.vector.tensor_add(query_sb[:], q_ps[:], sb["att_hb_col"][:])

            # ---- per-d-chunk psum groups [128 d, 1024 pos]: trans identity
            # is gather-INDEPENDENT, issued during the alpha DMA round-trip to
            # keep the PE warm; the K=121 coverage matmul lands after the
            # gather. Chunks 0-2 pre-issued (3 psum ring slots); chunk 3's
            # group is emitted after tanh(0) so its WAR on the recycled slot
            # cannot block conv(0). The lnmask energy term is window work too.
            energy_sb = sm(128, NJ)
            nc.tensor.matmul(
                energy_sb[:], ident_bf[:], sb["lnmask_col"][:],
                start=True, stop=False, skip_group_check=True,
            )

            def trans_mms(dc):
                cov_ps = big()
                for hf in range(2):
                    o0 = dc * HW + hf * 512
                    nc.tensor.matmul(
                        cov_ps[:, hf * 512 : hf * 512 + 512],
                        ident_bf[:],
                        sb["trans"][:, o0 : o0 + 512],
                        start=True,
                        stop=True,
                        skip_group_check=True,
                    )
                return cov_ps

            def convs(dc, cov_ps):
                for hf in range(2):
                    nc.tensor.matmul(
                        cov_ps[:, hf * 512 : hf * 512 + 512],
                        sb["k2"][:, dc * 128 : (dc + 1) * 128],
                        p2rep_v[:, hf * 8 : (hf + 1) * 8, 0:64],
                        start=False,
                        stop=True,
                        skip_group_check=True,
                    )

            def tanh_energy(dc, cov_ps):
                sc = scpool.tile([128, HW], BF, tag="sc")
                nc.scalar.activation(
                    sc[:], cov_ps[:], AF.Tanh, bias=query_sb[:, dc : dc + 1]
                )
                for j in range(NJ):
                    nc.tensor.matmul(
                        energy_sb[:, j : j + 1],
                        sc[:, j * 128 : (j + 1) * 128],
                        sb["w_col"][:, dc : dc + 1],
                        start=False,
                        stop=(dc == ND - 1 and j == NJ - 1),
                        skip_group_check=True,
                    )

            cov_list = [trans_mms(dc) for dc in range(3)]
            if t > 0:
                for dc in range(3):
                    convs(dc, cov_list[dc])
            tanh_energy(0, cov_list[0])
            cov3 = trans_mms(3)
            if t > 0:
                convs(3, cov3)
            for dc in range(1, 3):
                tanh_energy(dc, cov_list[dc])
            tanh_energy(3, cov3)

            # ---- softmax pieces (no max subtraction)
            esum = smpool.tile([128, 1], F32, tag="esum")
            nc.scalar.activation(
                e8_hist[:, NJ * t : NJ * (t + 1)],
                energy_sb[:],
                AF.Exp,
                bias=sb["ab_col"][:, 0:1],
                accum_out=esum[:],
            )
            sb_ps = sm(128, 1)
            nc.tensor.matmul(sb_ps[:], ones128_f32[:], esum[:], start=True, stop=True)
            nc.vector.reciprocal(rec_hist[:, t : t + 1], sb_ps[:])

            # alpha_sum is accumulated TRANSPOSED ([8 j, 128 (q,w)]) so the
            # scatter below is 16 contiguous 128B descriptors instead of a
            # partition-transposing (2-bytes-per-descriptor) write.
            if t < T - 1:
                e8t_ps = ps_small.tile([NJ, 128], BF, tag="sm", name="smps")
                nc.tensor.transpose(
                    e8t_ps[:], e8_hist[:, NJ * t : NJ * (t + 1)], ident_bf[:]
                )
                nc.vector.scalar_tensor_tensor(
                    alpha_t8[:],
                    e8t_ps[:],
                    rec_hist[0:NJ, t : t + 1],
                    alpha_t8[:],
                    op0=ALU.mult,
                    op1=ALU.add,
                )
                nc.vector.tensor_copy(alpha_f8[:], alpha_t8[:])
                nc.sync.dma_start(
                    bass.AP(p2d, 5 * PSTR + 5, [[2 * PSTR, NJ], [PSTR, 2], [1, 64]]),
                    alpha_f8[:],
                )
                p2rep = rpool.tile([121, GLEN], F8, tag="p2rep")
                nc.sync.dma_start(
                    p2rep[0:66, :],
                    bass.AP(p2d, 0, [[PSTR, 6], [1, 11], [1, GLEN]]),
                )
                nc.scalar.dma_start(
                    p2rep[66:121, :],
                    bass.AP(p2d, 6 * PSTR, [[PSTR, 5], [1, 11], [1, GLEN]]),
                )
                p2rep_v = p2rep[:].rearrange("k (h w) -> k h w", w=PSTR)

        # =================================================== batched tail
        os_bf = []
        for mc in range(2):
            ctx_ps = sm(128, T)
            for j in range(NJ):
                nc.tensor.matmul(
                    ctx_ps[:],
                    sb["m2t"][:, j * HID + mc * 128 : j * HID + (mc + 1) * 128],
                    e8v[:, j, :],
                    start=(j == 0),
                    stop=(j == NJ - 1),
                    skip_group_check=True,
                )
            os_ps = sm(128, T)
            for kc in range(2):
                nc.tensor.matmul(
                    os_ps[:],
                    sb["state_T"][:, kc * HID + mc * 128 : kc * HID + (mc + 1) * 128],
                    hbv[:, kc, 1 : T + 1],
                    start=(kc == 0),
                    stop=(kc == 1),
                    skip_group_check=True,
                )
            t1 = smpool.tile([128, T], F32, tag="tailt1")
            nc.vector.tensor_mul(t1[:], ctx_ps[:], rec_hist[:])
            t2 = smpool.tile([128, T], F32, tag="tailt2")
            nc.vector.tensor_add(t2[:], t1[:], sb["embw_pre"][:, mc * T : (mc + 1) * T])
            ob = smpool.tile([128, T], BF, tag="tailob")
            nc.vector.tensor_add(ob[:], t2[:], os_ps[:])
            os_bf.append(ob)

        pr_ps = sm(V, T)
        for kc in range(2):
            nc.tensor.matmul(
                pr_ps[:],
                sb["out_T"][:, kc * V : (kc + 1) * V],
                os_bf[kc][:],
                start=(kc == 0),
                stop=(kc == 1),
            )
        probs_sb = smpool.tile([V, T], F32, tag="probs")
        nc.vector.tensor_scalar_add(probs_sb[:], pr_ps[:], sb["out_b_col"][0:V, 0:1])

        # =================================================== epilogue
        pt_ps = sm(T, V)
        nc.tensor.transpose(pt_ps[:], probs_sb[:], ident[0:V, 0:V])
        out_sb = smpool.tile([T, V], F32, tag="outsb")
        nc.vector.tensor_copy(out_sb[:], pt_ps[:])
        nc.sync.dma_start(out_ext[:], out_sb[:])
        if dbg is not None:
            nc.sync.dma_start(dbg["dbg_e8"][:], e8_hist[:])
            nc.sync.dma_start(dbg["dbg_h"][:], hbf_hist[:])
            nc.sync.dma_start(dbg["dbg_rec"][:], rec_hist[:])


# ------------------------------------------------------------- host driver
def _prep_core_inputs(b, d, pos_all):
    g = lambda k: np.asarray(d[k], np.float32)
    cnn = g("cnn_features")[b].reshape(C, HW)
    mask = g("images_mask")[b, 0, ::RATIO, ::RATIO]
    dm = mask.reshape(-1)
    trans = (g("enc_conv_w")[:, :, 0, 0] @ cnn
             + pos_all[b].reshape(AD, HW) + g("enc_conv_b")[:, None])
    m2 = cnn.T @ g("ctx_W").T  # [HW, HID]
    words = np.concatenate([[1], np.asarray(d["labels"])[b, :-1].astype(np.int64)])
    we = g("emb")[words]  # [T, INP]
    gi = (g("gru_w_ih") @ we.T
          + (g("gru_b_ih")
             + np.concatenate([g("gru_b_hh")[:512], np.zeros(256, np.float32)]))[:, None])
    counting_ctx = g("count_W") @ g("counting_preds")[b] + g("count_b")
    embw = (g("embw_W") @ we.T
            + (g("state_b") + g("embw_b") + g("ctx_b") + counting_ctx)[:, None])
    avg = (cnn * dm[None, :]).sum(1) / dm.sum()
    h0 = np.tanh(g("init_W") @ avg + g("init_b"))
    return {
        "trans": _bf(_chunk_k(trans)),
        "m2t": _bf(_chunk_k(m2)),
        "gi_all": _f32(_chunk_k(gi)),
        "embw_pre": _f32(_chunk_k(embw)),
        "h0": _f32(_chunk_k(h0[:, None])),
        "lnmask_col": _f32(np.log(np.maximum(dm, 1e-30)).reshape(NJ, 128).T),
    }


def _prep_shared_inputs(d):
    g = lambda k: np.asarray(d[k], np.float32)
    return {
        "k2": _f8(g("att_conv_w").reshape(AD, 121).T @ g("att_weight_W").T),
        "w_hhT": _bf(_chunk_k(g("gru_w_hh").T)),
        "att_hT": _bf(_chunk_k(g("att_hidden_W").T)),
        "state_T": _bf(_chunk_k(g("state_W").T)),
        "out_T": _bf(_chunk_k(g("out_W").T)),
        "w_col": _bf(g("alpha_convert_W")[0].reshape(ND, 128).T),
        "att_hb_col": _f32(g("att_hidden_b").reshape(ND, 128).T),
        "bhn_col": _f32(g("gru_b_hh")[512:].reshape(2, 128).T),
        "out_b_col": _f32(np.pad(g("out_b"), (0, 128 - V))[:, None]),
        "ab_col": _f32(np.full((128, 1), float(g("alpha_convert_b")[0]))),
    }


_cached = {}


def kernel(**inputs) -> np.ndarray:
    if "nc" not in _cached:
        _cached["nc"] = build_kernel()
    nc = _cached["nc"]

    mask_hw = np.asarray(inputs["images_mask"], np.float32)[:, 0, ::RATIO, ::RATIO]
    pos_all = _pos_embedding_sine(mask_hw)
    shared = _prep_shared_inputs(inputs)
    in_maps = []
    for b in range(B):
        m = dict(shared)
        m.update(_prep_core_inputs(b, inputs, pos_all))
        in_maps.append(m)

    res = run_bass_kernel_spmd(nc, in_maps, core_ids=list(range(8)))
    out = np.stack([res.results[i]["out"] for i in range(8)], axis=0)
    return out.astype(np.float32)


if __name__ == "__main__":
    ins = dict(np.load("/root/problem/inputs.npz"))
    got = kernel(**ins)
    exp = np.load("/root/problem/expected.npy")
    rel = np.linalg.norm(got - exp) / np.linalg.norm(exp)
    print("Relative error:", rel)


# revision 26
# speedup vs baseline: 2.1994x; 1.0162x over previous
"""Trainium2 Bass kernel for nn_AttDecoder (GRU + coverage attention decoder).

Sharding: pure data parallel — batch 8 across 8 NeuronCores (batch=1/core).

v5 structure (host-validated numerics, rel err ~2.4e-3):
  - all static preprocessing on host: trans(+att_hidden_b) = enc_conv(cnn)+pos,
    K2 = att_weight_W @ att_conv_w, M2T = cnn^T ctx_W^T, gi, embw/bias terms,
    hidden0. Device loads ~2MB.
  - [d, pos] orientation: score psum chunks are [128 d, 1024 pos]. Per chunk
    the trans identity-matmul is gather-INDEPENDENT, issued during the alpha
    DMA round-trip to keep the PE warm (HAM); the K=121 coverage matmul from
    the one-DMA im2col gather lands after. tanh on ACT with query as the
    per-partition bias; energy via 32 small matmuls with lnmask as the
    accumulation group's window-issued first term.
  - alpha_sum accumulated TRANSPOSED ([8, 128] bf16 via a PE transpose of e8)
    so the scatter is 16 contiguous 128B descriptors, and the gather is split
    across the SP and ACT DMA queues.
  - ctx/out_state/prob tail batched AFTER the decode loop (N=36 matmuls).
Layouts: pos = h*64+w (1024); within 128-slices pos = j*128 + 64q + w with
h = 2j+q; d contiguous 512.
"""

import json
import math
import sys

import numpy as np
import ml_dtypes

sys.path.insert(0, "/opt/trn_rl_repo")

import concourse.bass as bass
import concourse.mybir as mybir
import concourse.tile as tile
from concourse.bass_utils import run_bass_kernel_spmd
from concourse.masks import make_identity

B, C, H, W = 8, 684, 16, 64
HID, INP, AD, V, T = 256, 256, 512, 111, 36
RATIO = 16
HW = H * W
ND = AD // 128
NJ = HW // 128
PSTR = 84
GLEN = 16 * PSTR  # im2col gather row length (16 h-rows x 84 stride)
P2D_LEN = 3072
BF = mybir.dt.bfloat16
F32 = mybir.dt.float32
F8 = mybir.dt.float8e4

_bf = lambda x: np.ascontiguousarray(np.asarray(x, dtype=np.float32)).astype(
    ml_dtypes.bfloat16
)
_f8 = lambda x: np.ascontiguousarray(np.asarray(x, dtype=np.float32)).astype(
    ml_dtypes.float8_e4m3
)
_f32 = lambda x: np.ascontiguousarray(np.asarray(x, dtype=np.float32))


def _chunk_k(a, k_pad=None):
    """[K, M] -> [128, (K/128)*M]; out[p, kc*M+m] = a[kc*128+p, m]."""
    a = np.asarray(a, dtype=np.float32)
    k, m = a.shape
    kp = k_pad or k
    if kp > k:
        a = np.concatenate([a, np.zeros((kp - k, m), np.float32)], 0)
    nk = kp // 128
    assert nk * 128 == kp
    return np.ascontiguousarray(
        a.reshape(nk, 128, m).transpose(1, 0, 2).reshape(128, nk * m)
    )


def _pos_embedding_sine(mask_hw):
    """numpy port of reference.pos_embedding_sine; [B,H,W] -> [B,512,H,W]."""
    num_pos_feats, temperature = 256, 10000.0
    scale = 2.0 * math.pi
    eps = 1e-6
    m = np.asarray(mask_hw, np.float32)
    y = np.cumsum(m, axis=1)
    x = np.cumsum(m, axis=2)
    y = y / (y[:, -1:, :] + eps) * scale
    x = x / (x[:, :, -1:] + eps) * scale
    i = np.arange(num_pos_feats, dtype=np.float32)
    dim_t = temperature ** (2.0 * np.floor(i / 2.0) / num_pos_feats)
    px = x[..., None] / dim_t
    py = y[..., None] / dim_t

    def inter(p):
        return np.stack((np.sin(p[..., 0::2]), np.cos(p[..., 1::2])), axis=4).reshape(
            p.shape[:3] + (num_pos_feats,)
        )

    pos = np.concatenate((inter(py), inter(px)), axis=3)
    return np.transpose(pos, (0, 3, 1, 2))


# ------------------------------------------------- walrus wait-split shim
def _split_sync_waits(bir_json: bytes, max_waits: int = 1) -> bytes:
    """This walrus build encodes one sem wait per instruction; hoist extras
    onto NoOps inserted before the instruction on the same engine."""
    js = json.loads(bir_json)
    n = 0
    for fn in js.get("functions", []):
        for bb in fn.get("blocks", []):
            out = []
            for ins in bb.get("instructions", []):
                si = ins.get("sync_info")
                waits = (si or {}).get("on_wait") or []
                upds = (si or {}).get("on_update") or []
                assert len(upds) <= 1, ins.get("name")
                if len(waits) > max_waits:
                    extra, si["on_wait"] = waits[:-max_waits], waits[-max_waits:]
                    for w in extra:
                        n += 1
                        out.append(
                            {
                                "debug": ins.get("debug", 0),
                                "engine": ins["engine"],
                                "ins": [],
                                "outs": [],
                                "name": f"WSPLIT-{n}",
                                "opcode": "NoOp",
                                "sync_info": {"on_wait": [w], "on_update": []},
                            }
                        )
                out.append(ins)
            bb["instructions"] = out
    return json.dumps(js).encode()


_shim_installed = False


def _install_shim():
    global _shim_installed
    if _shim_installed:
        return
    import concourse.bass2jax as bass2jax

    orig = bass2jax.compile_bir_kernel

    def wrapper(bir_json, tmpdir, neff_name="file.neff"):
        return orig(_split_sync_waits(bir_json), tmpdir, neff_name)

    bass2jax.compile_bir_kernel = wrapper
    _shim_installed = True


# ------------------------------------------------------------ bass builder
_INPUT_SPEC = {
    # small, needed first in the decode loop
    "gi_all": ([128, 6 * T], F32),
    "h0": ([128, 2], F32),
    "w_hhT": ([128, 2 * 3 * HID], BF),
    "att_hT": ([128, 2 * AD], BF),
    "lnmask_col": ([128, NJ], BF),
    "w_col": ([128, ND], BF),
    "bhn_col": ([128, 2], F32),
    "att_hb_col": ([128, ND], F32),
    "ab_col": ([128, 1], F32),
    "k2": ([121, AD], F8),
    # big per-core tensors
    "trans": ([128, ND * HW], BF),
    # tail-only
    "state_T": ([128, 2 * HID], BF),
    "out_T": ([128, 2 * V], BF),
    "embw_pre": ([128, 2 * T], F32),
    "out_b_col": ([128, 1], F32),
    "m2t": ([128, NJ * HID], BF),
}


def build_kernel(debug=False):
    _install_shim()
    nc = bass.Bass()
    dins = {
        k: nc.dram_tensor(k, s, d, kind="ExternalInput")
        for k, (s, d) in _INPUT_SPEC.items()
    }
    out_ext = nc.dram_tensor("out", [T, V], F32, kind="ExternalOutput")
    p2d = nc.dram_tensor("p2d", [P2D_LEN], F8)
    dbg = None
    if debug:
        dbg = {
            "dbg_e8": nc.dram_tensor("dbg_e8", [128, NJ * T], BF, kind="ExternalOutput"),
            "dbg_h": nc.dram_tensor("dbg_h", [128, 2 * (T + 1)], BF, kind="ExternalOutput"),
            "dbg_rec": nc.dram_tensor("dbg_rec", [128, T], F32, kind="ExternalOutput"),
        }
    with tile.TileContext(nc) as tc:
        _build_body(nc, tc, dins, out_ext, p2d, dbg)
    return nc


def _build_body(nc, tc, dins, out_ext, p2d, dbg=None):
    AF = mybir.ActivationFunctionType
    ALU = mybir.AluOpType

    with (
        tc.tile_pool(name="const", bufs=1) as cpool,
        tc.tile_pool(name="state", bufs=1) as spool,
        tc.tile_pool(name="p2rep", bufs=2) as rpool,
        tc.tile_pool(name="score", bufs=2) as scpool,
        tc.tile_pool(name="small", bufs=3) as smpool,
        tc.tile_pool(name="ps_big", bufs=3, space="PSUM") as ps_big,
        tc.tile_pool(name="ps_small", bufs=2, space="PSUM") as ps_small,
    ):
        big = lambda: ps_big.tile([128, HW], F32, tag="big", name="bigps")
        sm = lambda p_, f_: ps_small.tile([p_, f_], F32, tag="sm", name="smps")

        # ---- load all inputs to SBUF
        sb = {}
        for k, hndl in dins.items():
            t = cpool.tile(list(hndl.shape), hndl.dtype, tag=k)
            nc.sync.dma_start(t[:], hndl[:])
            sb[k] = t

        ident = cpool.tile([128, 128], F32, tag="ident")
        make_identity(nc, ident[:])
        ident_bf = cpool.tile([128, 128], BF, tag="ident_bf")
        nc.vector.tensor_copy(ident_bf[:], ident[:])
        ones128_f32 = cpool.tile([128, 128], F32, tag="ones128")
        nc.gpsimd.memset(ones128_f32[:], 1.0)

        # zero padded alpha staging buffer in DRAM
        zrow = cpool.tile([1, P2D_LEN], F8, tag="zrow")
        nc.gpsimd.memset(zrow[:], 0.0)
        nc.sync.dma_start(bass.AP(p2d, 0, [[P2D_LEN, 1], [1, P2D_LEN]]), zrow[:])

        # ---- persistent state / histories
        hidden = spool.tile([128, 2], F32, tag="hidden")
        alpha_t8 = spool.tile([NJ, 128], BF, tag="alpha_t8")
        alpha_f8 = spool.tile([NJ, 128], F8, tag="alpha_f8")
        e8_hist = spool.tile([128, NJ * T], BF, tag="e8_hist")
        hbf_hist = spool.tile([128, 2 * (T + 1)], BF, tag="hbf_hist")
        rec_hist = spool.tile([128, T], F32, tag="rec_hist")
        nc.gpsimd.memset(alpha_t8[:], 0.0)
        nc.vector.tensor_copy(hidden[:], sb["h0"][:])
        nc.vector.tensor_copy(hbf_hist[:, 0:2], sb["h0"][:])

        gi_view = sb["gi_all"][:].rearrange("p (m t) -> p t m", t=T)
        e8v = e8_hist[:].rearrange("p (t j) -> p j t", j=NJ)
        hbv = hbf_hist[:].rearrange("p (t k) -> p k t", k=2)

        # =================================================== decode loop
        p2rep_v = None
        for t in range(T):
            # ---- GRU (reads hbf_hist slice t = h(t-1))
            gh_ps = sm(128, 6)
            for mc in range(6):
                for kc in range(2):
                    nc.tensor.matmul(
                        gh_ps[:, mc : mc + 1],
                        sb["w_hhT"][:, kc * 768 + mc * 128 : kc * 768 + (mc + 1) * 128],
                        hbf_hist[:, 2 * t + kc : 2 * t + kc + 1],
                        start=(kc == 0 and mc == 0),
                        stop=(kc == 1 and mc == 5),
                        skip_group_check=True,
                    )
            rz_pre = smpool.tile([128, 4], F32, tag="rzpre")
            nc.vector.tensor_add(rz_pre[:], gh_ps[:, 0:4], gi_view[:, t, 0:4])
            rz_th = smpool.tile([128, 4], F32, tag="rzth")
            nc.scalar.activation(rz_th[:], rz_pre[:], AF.Tanh, scale=0.5)
            rz_sig = smpool.tile([128, 4], F32, tag="rzsig")
            nc.vector.tensor_scalar(
                rz_sig[:], rz_th[:], 0.5, 0.5, op0=ALU.mult, op1=ALU.add,
            )
            ghn_b = smpool.tile([128, 2], F32, tag="ghnb")
            nc.vector.tensor_add(ghn_b[:], gh_ps[:, 4:6], sb["bhn_col"][:])
            n_pre = smpool.tile([128, 2], F32, tag="npre")
            nc.vector.tensor_mul(n_pre[:], rz_sig[:, 0:2], ghn_b[:])
            n_pre2 = smpool.tile([128, 2], F32, tag="npre2")
            nc.vector.tensor_add(n_pre2[:], n_pre[:], gi_view[:, t, 4:6])
            n_sb = smpool.tile([128, 2], F32, tag="nsb")
            nc.scalar.activation(n_sb[:], n_pre2[:], AF.Tanh)
            hmn = smpool.tile([128, 2], F32, tag="hmn")
            nc.vector.tensor_sub(hmn[:], hidden[:], n_sb[:])
            zhm = smpool.tile([128, 2], F32, tag="zhm")
            nc.vector.tensor_mul(zhm[:], rz_sig[:, 2:4], hmn[:])
            nc.vector.tensor_add(hidden[:], n_sb[:], zhm[:])
            nc.vector.tensor_copy(hbf_hist[:, 2 * (t + 1) : 2 * (t + 2)], hidden[:])

            # ---- query [128, ND] (d on partitions within chunk, col = dc)
            q_ps = sm(128, ND)
            for mc in range(ND):
                for kc in range(2):
                    nc.tensor.matmul(
                        q_ps[:, mc : mc + 1],
                        sb["att_hT"][:, kc * AD + mc * 128 : kc * AD + (mc + 1) * 128],
                        hbf_hist[:, 2 * (t + 1) + kc : 2 * (t + 1) + kc + 1],
                        start=(kc == 0 and mc == 0),
                        stop=(kc == 1 and mc == ND - 1),
                        skip_group_check=True,
                    )
            query_sb = smpool.tile([128, ND], F32, tag="query")
            nc.vector.tensor_add(query_sb[:], q_ps[:], sb["att_hb_col"][:])

            # ---- per-d-chunk psum groups [128 d, 1024 pos]: trans identity
            # is gather-INDEPENDENT, issued during the alpha DMA round-trip to
            # keep the PE warm; the K=121 coverage matmul lands after the
            # gather. Chunks 0-2 pre-issued (3 psum ring slots); chunk 3's
            # group is emitted after tanh(0) so its WAR on the recycled slot
            # cannot block conv(0). The lnmask energy term is window work too.
            energy_sb = sm(128, NJ)
            nc.tensor.matmul(
                energy_sb[:], ident_bf[:], sb["lnmask_col"][:],
                start=True, stop=False, skip_group_check=True,
            )

            def trans_mms(dc):
                cov_ps = big()
                for hf in range(2):
                    o0 = dc * HW + hf * 512
                    nc.tensor.matmul(
                        cov_ps[:, hf * 512 : hf * 512 + 512],
                        ident_bf[:],
                        sb["trans"][:, o0 : o0 + 512],
                        start=True,
                        stop=True,
                        skip_group_check=True,
                    )
                return cov_ps

            def convs(dc, cov_ps):
                for hf in range(2):
                    nc.tensor.matmul(
                        cov_ps[:, hf * 512 : hf * 512 + 512],
                        sb["k2"][:, dc * 128 : (dc + 1) * 128],
                        p2rep_v[:, hf * 8 : (hf + 1) * 8, 0:64],
                        start=False,
                        stop=True,
                        skip_group_check=True,
                    )

            def tanh_energy(dc, cov_ps, halves=False):
                for hf in (0, 1) if halves else (0,):
                    w_half = 512 if halves else HW
                    sc = scpool.tile(
                        [128, w_half], BF, tag="sch" if halves else "sc"
                    )
                    nc.scalar.activation(
                        sc[:],
                        cov_ps[:, hf * 512 : hf * 512 + w_half],
                        AF.Tanh,
                        bias=query_sb[:, dc : dc + 1],
                    )
                    for jl in range(4 if halves else NJ):
                        j = hf * 4 + jl
                        nc.tensor.matmul(
                            energy_sb[:, j : j + 1],
                            sc[:, jl * 128 : (jl + 1) * 128],
                            sb["w_col"][:, dc : dc + 1],
                            start=False,
                            stop=(dc == ND - 1 and j == NJ - 1),
                            skip_group_check=True,
                        )

            cov_list = [trans_mms(dc) for dc in range(3)]
            if t > 0:
                for dc in range(3):
                    convs(dc, cov_list[dc])
            tanh_energy(0, cov_list[0])
            cov3 = trans_mms(3)
            if t > 0:
                convs(3, cov3)
            for dc in range(1, 3):
                tanh_energy(dc, cov_list[dc])
            tanh_energy(3, cov3, halves=True)

            # ---- softmax pieces (no max subtraction)
            esum = smpool.tile([128, 1], F32, tag="esum")
            nc.scalar.activation(
                e8_hist[:, NJ * t : NJ * (t + 1)],
                energy_sb[:],
                AF.Exp,
                bias=sb["ab_col"][:, 0:1],
                accum_out=esum[:],
            )
            sb_ps = sm(128, 1)
            nc.tensor.matmul(sb_ps[:], ones128_f32[:], esum[:], start=True, stop=True)
            nc.vector.reciprocal(rec_hist[:, t : t + 1], sb_ps[:])

            # alpha_sum is accumulated TRANSPOSED ([8 j, 128 (q,w)]) so the
            # scatter below is 16 contiguous 128B descriptors instead of a
            # partition-transposing (2-bytes-per-descriptor) write.
            if t < T - 1:
                e8t_ps = ps_small.tile([NJ, 128], BF, tag="sm", name="smps")
                nc.tensor.transpose(
                    e8t_ps[:], e8_hist[:, NJ * t : NJ * (t + 1)], ident_bf[:]
                )
                nc.vector.scalar_tensor_tensor(
                    alpha_f8[:],
                    e8t_ps[:],
                    rec_hist[0:NJ, t : t + 1],
                    alpha_t8[:],
                    op0=ALU.mult,
                    op1=ALU.add,
                )
                nc.sync.dma_start(
                    bass.AP(p2d, 5 * PSTR + 5, [[2 * PSTR, NJ], [PSTR, 2], [1, 64]]),
                    alpha_f8[:],
                )
                p2rep = rpool.tile([121, GLEN], F8, tag="p2rep")
                nc.sync.dma_start(
                    p2rep[:],
                    bass.AP(p2d, 0, [[PSTR, 11], [1, 11], [1, GLEN]]),
                )
                p2rep_v = p2rep[:].rearrange("k (h w) -> k h w", w=PSTR)
                # bf16 accumulator update, off the critical path
                nc.vector.scalar_tensor_tensor(
                    alpha_t8[:],
                    e8t_ps[:],
                    rec_hist[0:NJ, t : t + 1],
                    alpha_t8[:],
                    op0=ALU.mult,
                    op1=ALU.add,
                )

        # =================================================== batched tail
        os_bf = []
        for mc in range(2):
            ctx_ps = sm(128, T)
            for j in range(NJ):
                nc.tensor.matmul(
                    ctx_ps[:],
                    sb["m2t"][:, j * HID + mc * 128 : j * HID + (mc + 1) * 128],
                    e8v[:, j, :],
                    start=(j == 0),
                    stop=(j == NJ - 1),
                    skip_group_check=True,
                )
            os_ps = sm(128, T)
            for kc in range(2):
                nc.tensor.matmul(
                    os_ps[:],
                    sb["state_T"][:, kc * HID + mc * 128 : kc * HID + (mc + 1) * 128],
                    hbv[:, kc, 1 : T + 1],
                    start=(kc == 0),
                    stop=(kc == 1),
                    skip_group_check=True,
                )
            t1 = smpool.tile([128, T], F32, tag="tailt1")
            nc.vector.tensor_mul(t1[:], ctx_ps[:], rec_hist[:])
            t2 = smpool.tile([128, T], F32, tag="tailt2")
            nc.vector.tensor_add(t2[:], t1[:], sb["embw_pre"][:, mc * T : (mc + 1) * T])
            ob = smpool.tile([128, T], BF, tag="tailob")
            nc.vector.tensor_add(ob[:], t2[:], os_ps[:])
            os_bf.append(ob)

        pr_ps = sm(V, T)
        for kc in range(2):
            nc.tensor.matmul(
                pr_ps[:],
                sb["out_T"][:, kc * V : (kc + 1) * V],
                os_bf[kc][:],
                start=(kc == 0),
                stop=(kc == 1),
            )
        probs_sb = smpool.tile([V, T], F32, tag="probs")
        nc.vector.tensor_scalar_add(probs_sb[:], pr_ps[:], sb["out_b_col"][0:V, 0:1])

        # =================================================== epilogue
        pt_ps = sm(T, V)
        nc.tensor.transpose(pt_ps[:], probs_sb[:], ident[0:V, 0:V])
        out_sb = smpool.tile([T, V], F32, tag="outsb")
        nc.vector.tensor_copy(out_sb[:], pt_ps[:])
        nc.sync.dma_start(out_ext[:], out_sb[:])
        if dbg is not None:
            nc.sync.dma_start(dbg["dbg_e8"][:], e8_hist[:])
            nc.sync.dma_start(dbg["dbg_h"][:], hbf_hist[:])
            nc.sync.dma_start(dbg["dbg_rec"][:], rec_hist[:])


# ------------------------------------------------------------- host driver
def _prep_core_inputs(b, d, pos_all):
    g = lambda k: np.asarray(d[k], np.float32)
    cnn = g("cnn_features")[b].reshape(C, HW)
    mask = g("images_mask")[b, 0, ::RATIO, ::RATIO]
    dm = mask.reshape(-1)
    trans = (g("enc_conv_w")[:, :, 0, 0] @ cnn
             + pos_all[b].reshape(AD, HW) + g("enc_conv_b")[:, None])
    m2 = cnn.T @ g("ctx_W").T  # [HW, HID]
    words = np.concatenate([[1], np.asarray(d["labels"])[b, :-1].astype(np.int64)])
    we = g("emb")[words]  # [T, INP]
    gi = (g("gru_w_ih") @ we.T
          + (g("gru_b_ih")
             + np.concatenate([g("gru_b_hh")[:512], np.zeros(256, np.float32)]))[:, None])
    counting_ctx = g("count_W") @ g("counting_preds")[b] + g("count_b")
    embw = (g("embw_W") @ we.T
            + (g("state_b") + g("embw_b") + g("ctx_b") + counting_ctx)[:, None])
    avg = (cnn * dm[None, :]).sum(1) / dm.sum()
    h0 = np.tanh(g("init_W") @ avg + g("init_b"))
    return {
        "trans": _bf(_chunk_k(trans)),
        "m2t": _bf(_chunk_k(m2)),
        "gi_all": _f32(_chunk_k(gi)),
        "embw_pre": _f32(_chunk_k(embw)),
        "h0": _f32(_chunk_k(h0[:, None])),
        "lnmask_col": _f32(np.log(np.maximum(dm, 1e-30)).reshape(NJ, 128).T),
    }


def _prep_shared_inputs(d):
    g = lambda k: np.asarray(d[k], np.float32)
    return {
        "k2": _f8(g("att_conv_w").reshape(AD, 121).T @ g("att_weight_W").T),
        "w_hhT": _bf(_chunk_k(g("gru_w_hh").T)),
        "att_hT": _bf(_chunk_k(g("att_hidden_W").T)),
        "state_T": _bf(_chunk_k(g("state_W").T)),
        "out_T": _bf(_chunk_k(g("out_W").T)),
        "w_col": _bf(g("alpha_convert_W")[0].reshape(ND, 128).T),
        "att_hb_col": _f32(g("att_hidden_b").reshape(ND, 128).T),
        "bhn_col": _f32(g("gru_b_hh")[512:].reshape(2, 128).T),
        "out_b_col": _f32(np.pad(g("out_b"), (0, 128 - V))[:, None]),
        "ab_col": _f32(np.full((128, 1), float(g("alpha_convert_b")[0]))),
    }


_cached = {}


def kernel(**inputs) -> np.ndarray:
    if "nc" not in _cached:
        _cached["nc"] = build_kernel()
    nc = _cached["nc"]

    mask_hw = np.asarray(inputs["images_mask"], np.float32)[:, 0, ::RATIO, ::RATIO]
    pos_all = _pos_embedding_sine(mask_hw)
    shared = _prep_shared_inputs(inputs)
    in_maps = []
    for b in range(B):
        m = dict(shared)
        m.update(_prep_core_inputs(b, inputs, pos_all))
        in_maps.append(m)

    res = run_bass_kernel_spmd(nc, in_maps, core_ids=list(range(8)))
    out = np.stack([res.results[i]["out"] for i in range(8)], axis=0)
    return out.astype(np.float32)


if __name__ == "__main__":
    ins = dict(np.load("/root/problem/inputs.npz"))
    got = kernel(**ins)
    exp = np.load("/root/problem/expected.npy")
    rel = np.linalg.norm(got - exp) / np.linalg.norm(exp)
    print("Relative error:", rel)
